# revision 16
# baseline (speedup 1.0000x reference)
"""Trainium2 Bass kernel for nn_ExtractModel (retrieval_knn, soft edit-distance DP).

Strategy (8 NeuronCores, SPMD, vocab axis NT sharded):
  - Host: build index/one-hot tensors (fp16), shard vocab 8 ways sorted by
    vocab_length with uniform group boundaries across cores.
  - Device per core:
      1. char stage: word/unit representations + cosine via PE matmuls
         (multi-hot count matmul == embedding gather+sum), producing
         CT_adj[u, s] = cos_distance(s_char, u) - 2 (g-space constant fold).
      2. banded edit-distance DP in g-space (g = f - ls - lt, removes +1s):
         per cell (ls, lt): PSUM = one-hot gather matmul (diff) + identity
         matmul (adds prev state row); ScalarE drains PSUM->SBUF fp16;
         VectorE does q-min and prefix-min.  Vocab sorted by length =>
         per-cell suffix trimming.
      3. per (ls, lv) group min-reduce (GpSimd) -> [128, SCH] partials.
  - Host: min over cores, add (ls+lv) offsets, exact reference scoring.
"""

import numpy as np

# ---- problem constants (nn_ExtractModel spec) ----
MIN_WL, MAX_WL = 4, 10
MSL, MTL = 10, 10
THRESHOLD = 0.05
B, L, NT, U, G, NF, D = 8, 64, 8000, 64, 6, 512, 256
LEN_E = MAX_WL + 1 - MIN_WL
N_CORES = 8
P = 128


def _build_plan(lengths, vocab_length):
    lengths = np.asarray(lengths).astype(np.int64)
    vl = np.asarray(vocab_length).astype(np.int64)

    # --- source slot packing: (b, l) with l <= len_b - MIN_WL, b-major ---
    slot_b, slot_l, slot_base = [], [], []
    for b in range(B):
        slot_base.append(len(slot_b))
        run = max(0, int(lengths[b]) - (MIN_WL - 1))  # l in [0, len_b-4]
        slot_b += [b] * run
        slot_l += list(range(run))
    nslot = len(slot_b)
    sch = max(1, (nslot + P - 1) // P)
    slotpad = sch * P

    # --- vocab shard: sort by length, strided deal, pad groups uniform ---
    order = np.argsort(vl, kind="stable")
    cores_idx = [[] for _ in range(N_CORES)]
    gsizes = []
    for lv in range(MIN_WL, MAX_WL + 1):
        ids = order[vl[order] == lv]
        m = (len(ids) + N_CORES - 1) // N_CORES if len(ids) else 0
        gsizes.append(m)
        for c in range(N_CORES):
            share = list(ids[c::N_CORES]) if len(ids) else []
            if m:
                pad = share[0] if share else int(ids[0])
                share = (share + [pad] * m)[:m]
            cores_idx[c] += share
    nts = sum(gsizes)
    gs = np.zeros(MAX_WL + 2, dtype=np.int64)  # gs[lv] start of group lv
    acc = 0
    for i, lv in enumerate(range(MIN_WL, MAX_WL + 1)):
        gs[lv] = acc
        acc += gsizes[i]
    gs[MAX_WL + 1] = acc
    group_ok = [gsizes[lv - MIN_WL] > 0 for lv in range(MIN_WL, MAX_WL + 1)]

    # --- DP cell list ---
    cells = []
    for ls in range(1, MSL + 1):
        for lt in range(max(ls - 2, 1), min(ls + 2, MTL + 1)):
            t = min(max(lt, ls - 2, MIN_WL), MAX_WL)
            cells.append(dict(
                ls=ls, lt=lt, j=lt - 1,
                t0=int(gs[t]),
                ident=(ls >= 2 and lt - 1 >= max(ls - 3, 1)),
                qmin=(ls >= 2 and lt <= ls),
                prefix=(lt > max(ls - 2, 1)),
            ))

    # --- reduce pairs (ls, lv) ---
    pairs = []
    for ls in range(MIN_WL, MSL + 1):
        for lv in range(max(ls - 2, MIN_WL), min(ls + 1, MAX_WL) + 1):
            if group_ok[lv - MIN_WL]:
                pairs.append((ls, lv))

    return dict(
        lengths=lengths, slot_b=np.array(slot_b), slot_l=np.array(slot_l),
        slot_base=slot_base, nslot=nslot, sch=sch, slotpad=slotpad,
        cores_idx=cores_idx, nts=nts, gs=gs, cells=cells, pairs=pairs,
    )


def _build_host_tensors(plan, emb, feat_matrix, unit_feat_matrix,
                        indexed_segments):
    """Per-core in_maps (bf16 index/count tensors; float math on device)."""
    import ml_dtypes
    bf16 = ml_dtypes.bfloat16
    lengths = plan["lengths"]
    emb16 = np.asarray(emb).astype(bf16)
    emb_t = np.ascontiguousarray(emb16.reshape(4, P, D).transpose(1, 0, 2))

    # multi-hot count matrix cnt[nf, s=64b+p], zeroed at padded positions
    cnt = np.zeros((NF, B * L), dtype=np.float32)
    fm = np.asarray(feat_matrix).astype(np.int64)
    for b in range(B):
        lb = int(lengths[b])
        idx = fm[b, :lb, :].reshape(-1)                    # [lb*G]
        cols = np.repeat(b * L + np.arange(lb), G)
        np.add.at(cnt, (idx, cols), np.float32(1.0))
    cnt_t = np.ascontiguousarray(
        cnt.reshape(4, P, B * L).transpose(1, 0, 2)).astype(bf16)

    ucnt = np.zeros((NF, U), dtype=np.float32)
    um = np.asarray(unit_feat_matrix).astype(np.int64)
    np.add.at(ucnt, (um.reshape(-1), np.repeat(np.arange(U), G)),
              np.float32(1.0))
    ucnt_t = np.ascontiguousarray(
        ucnt.reshape(4, P, U).transpose(1, 0, 2)).astype(bf16)

    ident = np.eye(P, dtype=np.float32).astype(bf16)

    seg = np.asarray(indexed_segments).astype(np.int64)
    nts = plan["nts"]
    in_maps = []
    for c in range(N_CORES):
        ids = np.array(plan["cores_idx"][c], dtype=np.int64)
        segc = seg[ids]  # [nts, 10]
        Gt = np.zeros((U, MTL, nts), dtype=bf16)
        for j in range(MTL):
            Gt[segc[:, j], j, np.arange(nts)] = bf16(1.0)
        in_maps.append(dict(emb_w=emb_t, cnt_w=cnt_t, ucnt_w=ucnt_t,
                            ident_w=ident, g_w=Gt))
    return in_maps


def _build_nc(plan):
    import concourse.bass as bass
    import concourse.mybir as mybir
    from concourse.tile import TileContext

    fp16 = mybir.dt.bfloat16
    fp32 = mybir.dt.float32
    MIN = mybir.AluOpType.min
    MULT = mybir.AluOpType.mult
    ADD = mybir.AluOpType.add
    AX = mybir.AxisListType.X
    ACopy = mybir.ActivationFunctionType.Copy

    nts, sch, slotpad = plan["nts"], plan["sch"], plan["slotpad"]
    gs, cells, pairs = plan["gs"], plan["cells"], plan["pairs"]
    lengths = plan["lengths"]
    npair = len(pairs)
    S = B * L
    CTW = S + 16  # CT padded tail for shifted reads

    nc = bass.Bass()
    emb_w = nc.dram_tensor("emb_w", [P, 4, D], fp16, kind="ExternalInput")
    cnt_w = nc.dram_tensor("cnt_w", [P, 4, S], fp16, kind="ExternalInput")
    ucnt_w = nc.dram_tensor("ucnt_w", [P, 4, U], fp16, kind="ExternalInput")
    ident_w = nc.dram_tensor("ident_w", [P, P], fp16, kind="ExternalInput")
    g_w = nc.dram_tensor("g_w", [U, MTL, nts], fp16,
                         kind="ExternalInput")
    m_out = nc.dram_tensor("m_out", [P, npair * sch], fp16,
                           kind="ExternalOutput")

    with TileContext(nc) as tc:
        with (
            tc.tile_pool(name="inp", bufs=1) as inp,
            tc.tile_pool(name="work", bufs=1) as work,
            tc.tile_pool(name="state", bufs=1) as stp,
        ):
            # ---- load inputs ----
            emb_s = inp.tile([P, 4, D], fp16, tag="emb")
            cnt_s = inp.tile([P, 4, S], fp16, tag="cnt")
            ucnt_s = inp.tile([P, 4, U], fp16, tag="ucnt")
            ident_s = inp.tile([P, P], fp16, tag="ident")
            g_s = inp.tile([U, MTL, nts], fp16, tag="g")
            nc.sync.dma_start(emb_s[:], emb_w[:])
            nc.sync.dma_start(cnt_s[:], cnt_w[:])
            nc.sync.dma_start(ucnt_s[:], ucnt_w[:])
            nc.sync.dma_start(ident_s[:], ident_w[:])
            nc.sync.dma_start(g_s[:], g_w[:])

            ones_s = work.tile([P, 1], fp16, tag="ones")
            nc.vector.memset(ones_s[:], 1.0)

            wrT = work.tile([P, 2, S], fp16, tag="wrT")      # [d-part, m, s]
            wsq = work.tile([P, 2, S], fp16, tag="wsq")
            unT = work.tile([P, 2, U], fp16, tag="unT")      # [d-part, m, u]
            usq = work.tile([P, 2, U], fp16, tag="usq")
            unN_ud = work.tile([U, 2, P], fp16, tag="unN_ud")
            unNT = work.tile([P, 2, U], fp16, tag="unNT")    # ry-scaled [d,m,u]
            rx_s = work.tile([P, 4], fp32, tag="rx")         # -0.5/nx per s
            ry_s = work.tile([U, 1], fp32, tag="ry")         # 1/ny per u
            tmp_s = work.tile([P, 4], fp32, tag="tmpx")
            tmpy = work.tile([U, 1], fp32, tag="tmpy")
            dotn = work.tile([P, 4, U], fp16, tag="dotn")    # [s-part, sc, u]
            ct = work.tile([U, CTW], fp16, tag="ct")
            cs = [work.tile([U, slotpad], fp16, tag=f"cs{ls}", name=f"cs{ls}")
                  for ls in range(1, MSL + 1)]
            mtile = work.tile([P, npair * sch], fp16, tag="mt")

            with tc.tile_pool(name="cpsum", bufs=1, space="PSUM") as cp:
                # word representations: wrT[d, s] = emb.T @ cnt
                for m in range(2):
                    ps = cp.tile([P, S], fp32, tag="cps", name="cps")
                    for k in range(4):
                        nc.tensor.matmul(
                            ps[:], emb_s[:, k, m * P:(m + 1) * P],
                            cnt_s[:, k, :], start=(k == 0), stop=(k == 3))
                    nc.scalar.copy(wrT[:, m, :], ps[:])
                # unit representations
                for m in range(2):
                    pu = cp.tile([P, U], fp32, tag="cpu", name="cpu")
                    for k in range(4):
                        nc.tensor.matmul(
                            pu[:], emb_s[:, k, m * P:(m + 1) * P],
                            ucnt_s[:, k, :], start=(k == 0), stop=(k == 3))
                    nc.scalar.copy(unT[:, m, :], pu[:])

                # squared norms via ones-matmul (per-partition orientation)
                nc.vector.tensor_tensor(wsq[:], wrT[:], wrT[:], op=MULT)
                nc.vector.tensor_tensor(usq[:], unT[:], unT[:], op=MULT)
                pnx = cp.tile([P, 4], fp32, tag="pnx")
                for sc_ in range(4):
                    for m in range(2):
                        nc.tensor.matmul(
                            pnx[:, sc_:sc_ + 1],
                            wsq[:, m, sc_ * P:(sc_ + 1) * P], ones_s[:],
                            start=(m == 0), stop=(m == 1))
                pny = cp.tile([U, 1], fp32, tag="pny")
                for m in range(2):
                    nc.tensor.matmul(pny[:], usq[:, m, :], ones_s[:],
                                     start=(m == 0), stop=(m == 1))

                # rx = -0.5 / (||wr|| + 1e-8); ry = 1 / (||un|| + 1e-8)
                nc.scalar.sqrt(tmp_s[:], pnx[:])
                nc.vector.tensor_scalar_add(tmp_s[:], tmp_s[:], 1e-8)
                nc.vector.reciprocal(rx_s[:], tmp_s[:])
                nc.vector.tensor_scalar_mul(rx_s[:], rx_s[:], -0.5)
                nc.scalar.sqrt(tmpy[:], pny[:])
                nc.vector.tensor_scalar_add(tmpy[:], tmpy[:], 1e-8)
                nc.vector.reciprocal(ry_s[:], tmpy[:])

                # unNT = (unit/||unit||)^T back to [d, u] layout
                for m in range(2):
                    pt = cp.tile([U, P], fp16, tag="ptr", name="ptr")
                    nc.tensor.transpose(pt[:], unT[:, m, :], ident_s[:])
                    nc.vector.tensor_scalar(unN_ud[:, m, :], pt[:],
                                            ry_s[:], None, op0=MULT)
                for m in range(2):
                    pt2 = cp.tile([P, U], fp16, tag="ptr2", name="ptr2")
                    nc.tensor.transpose(pt2[:], unN_ud[:, m, :],
                                        ident_s[:U, :U])
                    nc.scalar.copy(unNT[:, m, :], pt2[:])

                # dotn[s, u] = -0.5 * cos_n(s, u)
                for sc_ in range(4):
                    pd = cp.tile([P, U], fp32, tag="pd", name="pd")
                    for m in range(2):
                        nc.tensor.matmul(
                            pd[:], wrT[:, m, sc_ * P:(sc_ + 1) * P],
                            unNT[:, m, :], start=(m == 0), stop=(m == 1))
                    nc.vector.tensor_scalar(
                        dotn[:, sc_, :], pd[:], rx_s[:, sc_:sc_ + 1], None,
                        op0=MULT)
                # CT[u, s] = dotn^T + 0.5  (= cos distance, f-space)
                for sc_ in range(4):
                    pt3 = cp.tile([U, P], fp16, tag="pt3", name="pt3")
                    nc.tensor.transpose(pt3[:], dotn[:, sc_, :], ident_s[:])
                    nc.scalar.activation(ct[:, sc_ * P:(sc_ + 1) * P], pt3[:],
                                         ACopy, bias=0.5, scale=1.0)

            nc.gpsimd.memset(ct[:, S:], 0.0)

            # ---- shifted packed char matrices CS_ls[u, slot] ----
            for lsi in range(MSL):
                nc.gpsimd.memset(cs[lsi][:], 0.0)
                for b in range(B):
                    run = max(0, int(lengths[b]) - (MIN_WL - 1))
                    if run == 0:
                        continue
                    sb = plan["slot_base"][b]
                    nc.vector.tensor_copy(
                        cs[lsi][:, sb:sb + run],
                        ct[:, b * L + lsi: b * L + lsi + run])

            # ---- DP state: tiles per (parity, row mod 6) ----
            st = [[stp.tile([P, sch, nts], fp16, tag=f"st{par}_{r}",
                            name=f"st{par}_{r}")
                   for r in range(6)] for par in range(2)]

            pair_idx = {pq: i for i, pq in enumerate(pairs)}
            tmpt = [stp.tile([P, sch, nts], fp16, tag=f"tmp{i}",
                             name=f"tmp{i}") for i in range(3)]
            load = {"dve": 0.0, "gp": 0.0, "pe": 0.0, "act": 0.0}
            tctr = [0]

            def _tt(eng, out, a, b):
                (nc.vector if eng == "dve" else nc.gpsimd).tensor_tensor(
                    out, a, b, op=MIN)

            with tc.tile_pool(name="dpsum", bufs=4, space="PSUM") as dp:
                for cell in cells:
                    ls, lt, j, t0 = (cell["ls"], cell["lt"], cell["j"],
                                     cell["t0"])
                    w = nts - t0
                    if w <= 0:
                        continue
                    par = ls % 2
                    cur = st[par][lt % 6]
                    prevrow = st[1 - par][lt % 6]
                    prevm1 = st[1 - par][(lt - 1) % 6]
                    curm1 = st[par][(lt - 1) % 6]
                    gsl = g_s[:, j, :]
                    a_real = (ls >= 2 and lt <= ls)
                    b_real = (lt - 1 >= max(ls - 2, 1))
                    if cell["ident"]:
                        dbias = 0.0
                    elif ls == 1:
                        dbias = float(lt - 1)
                    else:
                        dbias = float(ls - 1)
                    pe_id = sch * (248 + 0.833 * w)
                    act_dr = sch * (143 + 0.833 * w)
                    dve_st = sch * (125 + 1.042 * w)
                    m1 = max(load["pe"] + pe_id, load["act"] + act_dr,
                             load["dve"])
                    m2 = max(load["pe"], load["act"], load["dve"] + dve_st)
                    use_stt = cell["ident"] and m2 < m1
                    for c in range(sch):
                        ps = dp.tile([P, 1024], fp32, tag="dps", name="dps")
                        for n0 in range(0, w, 512):
                            n1 = min(n0 + 512, w)
                            nc.tensor.matmul(
                                ps[:, n0:n1],
                                cs[ls - 1][:, c * P:(c + 1) * P],
                                gsl[:, t0 + n0:t0 + n1], start=True,
                                stop=(not cell["ident"]) or use_stt)
                            load["pe"] += 124 + 0.833 * (n1 - n0)
                            if cell["ident"] and not use_stt:
                                nc.tensor.matmul(
                                    ps[:, n0:n1], ident_s[:],
                                    prevm1[:, c, t0 + n0:t0 + n1],
                                    start=False, stop=True)
                                load["pe"] += 124 + 0.833 * (n1 - n0)
                        if use_stt:
                            nc.vector.scalar_tensor_tensor(
                                cur[:, c, t0:], prevm1[:, c, t0:], 0.0,
                                ps[:, 0:w], op0=ADD, op1=ADD)
                            load["dve"] += 125 + 1.042 * w
                        else:
                            nc.scalar.activation(cur[:, c, t0:], ps[:, 0:w],
                                                 ACopy, bias=dbias, scale=1.0)
                            load["act"] += 143 + 0.833 * w
                    fd = sch * w
                    rng = slice(t0, nts)
                    tb = tmpt[tctr[0] % 3][:, :, rng]
                    tctr[0] += 1
                    load["dve"] += fd * 1.042 + 240
                    if ls == 1:
                        if lt == 1:
                            nc.vector.tensor_scalar(
                                cur[:, :, rng], cur[:, :, rng], 2.0, None,
                                op0=MIN)
                        else:  # (1, 2): cur = min(c, (cur1 + 1) min 3)
                            nc.vector.tensor_scalar(
                                tb, st[par][1][:, :, rng], 1.0, 3.0,
                                op0=ADD, op1=MIN)
                            nc.vector.tensor_tensor(
                                cur[:, :, rng], cur[:, :, rng], tb, op=MIN)
                    elif a_real and b_real:
                        nc.vector.tensor_tensor(
                            tb, prevrow[:, :, rng], curm1[:, :, rng], op=MIN)
                        c_dve = (58 + fd / 4) * 1.042
                        c_act = 187 + fd * 0.833
                        if load["act"] + c_act < load["dve"] + c_dve:
                            load["act"] += c_act
                            for cc in range(sch):
                                nc.scalar.activation(
                                    tb[:, cc, :], tb[:, cc, :], ACopy,
                                    bias=1.0, scale=1.0)
                        else:
                            load["dve"] += c_dve
                            nc.vector.tensor_scalar(tb, tb, 1.0, None,
                                                    op0=ADD)
                        nc.vector.tensor_tensor(
                            cur[:, :, rng], cur[:, :, rng], tb, op=MIN)
                    elif a_real and lt == 1:  # b = const ls
                        nc.vector.tensor_scalar(
                            tb, prevrow[:, :, rng], 1.0, float(ls + 1),
                            op0=ADD, op1=MIN)
                        nc.vector.tensor_tensor(
                            cur[:, :, rng], cur[:, :, rng], tb, op=MIN)
                    else:
                        x = prevrow if a_real else curm1
                        c_dve = (58 + fd / 4) * 1.042
                        c_act = 187 + fd * 0.833
                        if load["act"] + c_act < load["dve"] + c_dve:
                            load["act"] += c_act
                            for cc in range(sch):
                                nc.scalar.activation(
                                    tb[:, cc, :], x[:, cc, rng], ACopy,
                                    bias=1.0, scale=1.0)
                        else:
                            load["dve"] += c_dve
                            nc.vector.tensor_scalar(tb, x[:, :, rng], 1.0,
                                                    None, op0=ADD)
                        nc.vector.tensor_tensor(
                            cur[:, :, rng], cur[:, :, rng], tb, op=MIN)

                    # group min-reduce once row lt is final for step ls
                    i = pair_idx.get((ls, lt))
                    if i is not None:
                        g0, g1 = int(gs[lt]), int(gs[lt + 1])
                        nc.vector.tensor_reduce(
                            mtile[:, i * sch:(i + 1) * sch],
                            cur[:, :, g0:g1], axis=AX, op=MIN)
                        load["dve"] += (58 + sch * (g1 - g0)) * 1.05

            nc.sync.dma_start(m_out[:], mtile[:])

    _split_waits(nc, cap=1)
    return nc


def _split_waits(nc, cap=1):
    """This walrus build accepts at most one sync-wait per instruction:
    hoist excess waits onto preceding same-engine NoOps (sequencer order
    preserves blocking semantics)."""
    import concourse.mybir as mybir
    import bass_rust

    for bb in nc.main_func.blocks:
        out = []
        for ins in bb.instructions:
            si = ins.sync_info
            if si is not None and len(si.on_wait) > cap:
                waits = list(si.on_wait)
                k = 0
                while len(waits) > cap:
                    chunk, waits = waits[:cap], waits[cap:]
                    nop = mybir.InstNoOp(name=f"{ins.name}_ws{k}", ins=[],
                                         outs=[])
                    nop.engine = ins.engine
                    nop.sync_info = bass_rust.SyncInfo(on_wait=chunk,
                                                       on_update=[])
                    nc.register_instruction(nop, overwrite=True)
                    out.append(nop)
                    k += 1
                ins.sync_info = bass_rust.SyncInfo(
                    on_wait=waits, on_update=list(si.on_update))
            out.append(ins)
        bb.instructions = out


def _postprocess(plan, results):
    sch, gs, pairs = plan["sch"], plan["gs"], plan["pairs"]
    lengths = plan["lengths"]
    nslot = plan["nslot"]
    slot_b, slot_l = plan["slot_b"], plan["slot_l"]

    m_all = np.stack([np.asarray(r["m_out"], dtype=np.float32)
                      for r in results], axis=0)  # [8, 128, npair*sch]
    m_min = m_all.min(axis=0)

    best = np.full((B, L, LEN_E), 99.9, dtype=np.float32)
    s_arr = np.arange(nslot)
    c_arr, p_arr = s_arr // P, s_arr % P
    for i, (ls, lv) in enumerate(pairs):
        f = m_min[:, i * sch:(i + 1) * sch]
        vals = f[p_arr, c_arr]  # [nslot]
        e = ls - MIN_WL
        cur = best[slot_b, slot_l, e]
        best[slot_b, slot_l, e] = np.minimum(cur, vals)

    pos = np.arange(L)
    len_cand = MIN_WL + np.arange(LEN_E)
    end_cand = pos[:, None] + len_cand[None, :] - 1
    viable = end_cand[None] < lengths[:, None, None]
    score = len_cand.astype(np.float32) * (np.float32(1.0) - best)
    score = np.where(viable, score, np.float32(0.0))
    matched = viable & (best < np.float32(THRESHOLD))
    flat = score.reshape(B, L * LEN_E)
    best_scores = flat.max(axis=-1)
    best_inds = flat.argmax(axis=-1)
    best_starts = best_inds // LEN_E
    best_ends = best_inds % LEN_E + best_starts + MIN_WL - 1
    matched_any = matched.reshape(B, -1).any(axis=-1)
    return (best_scores.astype(np.float32),
            best_starts.astype(np.int64),
            best_ends.astype(np.int64),
            matched_any)


def _install_profile_shim():
    """Provide antenv.axon_hooks (missing from this image) so that
    run_bass_kernel_spmd(trace=True) can capture NTFF profiles, and stub the
    remote artifact upload (no share access in this container)."""
    import sys
    import types

    if "antenv.axon_hooks" not in sys.modules:
        mod = types.ModuleType("antenv.axon_hooks")
        holder = [None]
        mod.set_axon_ntff_profile_hook = lambda h: holder.__setitem__(0, h)
        mod.get_axon_ntff_profile_hook = lambda: holder[0]
        sys.modules["antenv.axon_hooks"] = mod
        import antenv
        antenv.axon_hooks = mod
    from antenv import axon_hooks
    if axon_hooks.get_axon_ntff_profile_hook() is None:
        from trn_agent_boot.trn_boot import _ntff_profile_via_ctypes
        axon_hooks.set_axon_ntff_profile_hook(
            _ntff_profile_via_ctypes("/opt/axon/libaxon_pjrt.so"))
    from concourse import bass_utils
    bass_utils.upload_artifacts = lambda tmpdir: "local://" + str(tmpdir)


def kernel_with_info(emb, feat_matrix, lengths, unit_feat_matrix,
                     indexed_segments, vocab_length, trace=False):
    from concourse import bass_utils

    plan = _build_plan(lengths, vocab_length)
    in_maps = _build_host_tensors(plan, emb, feat_matrix, unit_feat_matrix,
                                  indexed_segments)
    nc = _build_nc(plan)
    if trace:
        try:
            _install_profile_shim()
            res = bass_utils.run_bass_kernel_spmd(
                nc, in_maps, core_ids=list(range(N_CORES)), trace=True)
        except Exception as e:
            print(f"[kernel] traced run failed ({type(e).__name__}: {e}); "
                  f"retrying without trace")
            res = bass_utils.run_bass_kernel_spmd(
                nc, in_maps, core_ids=list(range(N_CORES)), trace=False)
    else:
        res = bass_utils.run_bass_kernel_spmd(
            nc, in_maps, core_ids=list(range(N_CORES)), trace=False)
    outs = _postprocess(plan, res.results)
    return outs, res.exec_time_ns


def kernel(emb, feat_matrix, lengths, unit_feat_matrix, indexed_segments,
           vocab_length):
    outs, _ = kernel_with_info(emb, feat_matrix, lengths, unit_feat_matrix,
                               indexed_segments, vocab_length, trace=False)
    return outs
